# revision 26
# baseline (speedup 1.0000x reference)
"""Causal multi-head attention (B=4, T=2048, C=1024, H=16, HD=64) on 8 TRN2
NeuronCores.

Sharding: 2D — batch (4) x head-group (2 groups of 8 heads). Each core handles
one batch's tokens for 8 heads:
    core = b * 2 + g
    OC = 512 local channels; host-side reduce over g of yT.T partials.

v2 layout/schedule (vs. v1 baseline at ~342us):
  - All on-chip operands bf16 (x, Wq/Wk/Wv/Wo, q/k/v, ctx). PSUM stays f32;
    y output stays f32. Halves input DMA (16->8 MB per core).
  - Host pre-tiles every input into the exact SBUF layout with 8 KB
    contiguous rows -> 8 consolidated input DMAs with full-size packets.
  - Causal trim: per query block j, diagonal key chunk c=4j+t only streams
    q in [128t, 512) through the S^T and ctx matmuls (saves ~20us of PE
    streams); masking is a single 128x128 affine_select per diagonal chunk
    (keep f - p >= 0) instead of full-tile masks.
  - Ascending chunk order so ctx PSUM accumulation starts with the full-width
    chunk 0 (partial-width chunks then accumulate into already-started cols).
  - Filler rebalance: scalar-engine exp (1114ns per chunk pair) outruns the
    PE's attention matmuls (864ns per pair), so attend(j) interleaves spare
    matmul groups: proj(j+1) for j=0..2 and ALL output projections out(0..2)
    during attend(3) (which has the largest exp deficit). Filler is paced one
    unit per ~3 pairs to spread it across the exp-limited stretch.
  - Q^T/K^T in [channel, token] bf16; V in [token, channel] bf16 with an
    all-ones column per head so the ctx matmul also produces the softmax
    denominator row; DVE reciprocal + gpsimd partition_broadcast + DVE mult
    for normalization (deferred via a pending queue).
"""

import numpy as np

B, T_FULL, C = 4, 2048, 1024
H, HD = 16, 64
GROUPS = 2
HL = H // GROUPS          # heads per core = 8
OC = HL * HD              # local channels = 512
P = 128                   # partitions
TB = 512                  # token block (moving dim)
SCALE = float(1.0 / np.sqrt(HD))
NCORES = 8


def build_program(T=T_FULL):
    from contextlib import ExitStack

    import concourse.bacc as bacc
    import concourse.mybir as mybir
    import concourse.tile as tile

    f32 = mybir.dt.float32
    bf16 = mybir.dt.bfloat16
    EXP = mybir.ActivationFunctionType.Exp
    GE = mybir.AluOpType.is_ge
    MULT = mybir.AluOpType.mult

    NTB = T // TB             # 512-token blocks = 4
    NKC = T // P              # 128-token key chunks = 16
    CCH = C // P              # 8 contraction chunks of C
    MCH = OC // P             # 4 output-channel chunks

    nc = bacc.Bacc("TRN2", target_bir_lowering=False, debug=False)
    # Host-pretiled inputs: row p holds the SBUF partition-p contents.
    xh = nc.dram_tensor("xh", [P, NTB * CCH * TB], bf16, kind="ExternalInput").ap()
    wqh = nc.dram_tensor("wqh", [P, MCH * CCH * P], bf16, kind="ExternalInput").ap()
    wkh = nc.dram_tensor("wkh", [P, MCH * CCH * P], bf16, kind="ExternalInput").ap()
    wvh = nc.dram_tensor("wvh", [P, CCH * OC], bf16, kind="ExternalInput").ap()
    woh = nc.dram_tensor("woh", [P, MCH * C], bf16, kind="ExternalInput").ap()
    yT = nc.dram_tensor("yT", [C, T], f32, kind="ExternalOutput").ap()

    with tile.TileContext(nc) as tc, ExitStack() as ctx:
        perm = ctx.enter_context(tc.tile_pool(name="perm", bufs=1))
        psperm = ctx.enter_context(tc.tile_pool(name="psperm", bufs=1, space="PSUM"))

        # persistent SBUF tiles
        xb = [perm.tile([P, CCH * TB], bf16, tag=f"xb{t}", name=f"xb{t}")
              for t in range(NTB)]
        wqt = perm.tile([P, MCH * CCH * P], bf16, tag="wqt", name="wqt")
        wkt = perm.tile([P, MCH * CCH * P], bf16, tag="wkt", name="wkt")
        wvt = perm.tile([P, CCH * OC], bf16, tag="wvt", name="wvt")
        wot = perm.tile([P, MCH * C], bf16, tag="wot", name="wot")
        qt = [perm.tile([P, T], bf16, tag=f"qt{h}", name=f"qt{h}") for h in range(HL)]
        kt = [perm.tile([P, T], bf16, tag=f"kt{m}", name=f"kt{m}") for m in range(MCH)]
        ct = [perm.tile([P, T], bf16, tag=f"ct{m}", name=f"ct{m}") for m in range(MCH)]
        # V padded to 128 cols per head: [V_h | 1 | 0...] so ctx lhsT is M=128
        v = [perm.tile([P, HL * P], bf16, tag=f"v{t}", name=f"v{t}")
             for t in range(NKC)]
        # S^T psum tiles, manually rotated (memset once so trimmed-pair
        # activations never read never-written PSUM)
        stt = [psperm.tile([P, 2 * TB], f32, tag=f"st{i}", name=f"st{i}")
               for i in range(2)]

        # ---- input DMAs, priority order (wq split per-m so the first
        # projection group is gated on only x0 + 256 KB of weights) ----
        MW = CCH * P
        HB = CCH * TB // 2
        nc.sync.dma_start(out=xb[0][:, 0:HB], in_=xh[:, 0:HB])
        nc.sync.dma_start(out=wqt[:, 0:MW], in_=wqh[:, 0:MW])
        nc.sync.dma_start(out=xb[0][:, HB:2 * HB], in_=xh[:, HB:2 * HB])
        for m in range(1, MCH):
            nc.sync.dma_start(out=wqt[:, m * MW:(m + 1) * MW],
                              in_=wqh[:, m * MW:(m + 1) * MW])
        nc.sync.dma_start(out=wkt, in_=wkh)
        nc.sync.dma_start(out=wvt, in_=wvh)
        for t in range(1, NTB):
            nc.sync.dma_start(out=xb[t], in_=xh[:, t * CCH * TB:(t + 1) * CCH * TB])
        nc.sync.dma_start(out=wot, in_=woh)

        # ---- one-time memsets, all on gpsimd (its first real work — the
        # first affine_select — comes well after these drain), ordered by
        # first use so nothing blocks the projection pipeline's DVE copies.
        ONE_BF16 = 0x3F80  # 1.0 in bf16 — bf16 memset via uint16 bitcast
        for st in stt:
            nc.vector.memset(st, 0.0)  # gpsimd memset cannot target PSUM

        def memset_v(vt):
            vv = vt.rearrange("p (h e) -> p h e", e=P)
            nc.gpsimd.memset(vv[:, :, 64:65].bitcast(mybir.dt.uint16), ONE_BF16)
            nc.gpsimd.memset(vv[:, :, 65:].bitcast(mybir.dt.uint16), 0)

        for vt in v[:4]:
            memset_v(vt)
        for h in range(HL):
            z0 = (1 - h % 2) * 64  # zero rows: the other head's half
            nc.gpsimd.memset(qt[h][z0:z0 + 64, :].bitcast(mybir.dt.uint16), 0)
        for vt in v[4:]:
            memset_v(vt)

        with (
            tc.tile_pool(name="ptpool", bufs=4) as ptp,
            tc.tile_pool(name="tmppool", bufs=2) as tmp,
            tc.tile_pool(name="ypool", bufs=4) as yp,
            tc.tile_pool(name="mmps", bufs=2, space="PSUM") as pp,
            tc.tile_pool(name="ctxps", bufs=2, space="PSUM") as cxp,
        ):
            def proj_groups(tb):
                groups = []

                def proj_qk(wt, isq, m, tb=tb, copy_eng=None):
                    def go():
                        ps = pp.tile([P, TB], f32, tag="mm512",
                                     name=f"ps_{tb}_{m}_{isq}")
                        for c in range(CCH):
                            nc.tensor.matmul(
                                ps,
                                lhsT=wt[:, m * (CCH * P) + c * P:
                                        m * (CCH * P) + (c + 1) * P],
                                rhs=xb[tb][:, c * TB:(c + 1) * TB],
                                start=(c == 0), stop=(c == CCH - 1))
                        cp = (nc.scalar.copy if copy_eng == "scalar"
                              else nc.vector.tensor_copy)
                        if isq:
                            for hh in (0, 1):
                                r0_ = hh * 64
                                cp(qt[2 * m + hh][r0_:r0_ + 64,
                                                  tb * TB:(tb + 1) * TB],
                                   ps[r0_:r0_ + 64, :])
                        else:
                            cp(kt[m][:, tb * TB:(tb + 1) * TB], ps)
                    return go

                def proj_v(ts_, tb=tb, copy_eng=None):
                    def go():
                        ps = pp.tile([P, OC], f32, tag="mm512",
                                     name=f"psv_{tb}_{ts_}")
                        for c in range(CCH):
                            nc.tensor.matmul(
                                ps,
                                lhsT=xb[tb][:, c * TB + ts_ * P:
                                            c * TB + (ts_ + 1) * P],
                                rhs=wvt[:, c * OC:(c + 1) * OC],
                                start=(c == 0), stop=(c == CCH - 1))
                        ti = tb * (TB // P) + ts_
                        cp = (nc.scalar.copy if copy_eng == "scalar"
                              else nc.vector.tensor_copy)
                        cp(v[ti].rearrange("p (h e) -> p h e", e=P)[:, :, 0:64],
                           ps.rearrange("p (h d) -> p h d", d=64))
                    return go

                # the last 6 units are consumed at the attend tail where the
                # DVE queue is congested with norms — put their psum->sbuf
                # copies on the scalar engine (drained of exps by then)
                for m in range(MCH):
                    groups.append(proj_qk(wqt, True, m))
                for m in range(MCH):
                    groups.append(proj_qk(wkt, False, m,
                                          copy_eng="scalar" if m >= 2 else None))
                for ts_ in range(TB // P):
                    groups.append(proj_v(ts_, copy_eng="scalar"))
                return groups

            def out_groups(tb, copy_eng=None):
                def out_co(co, tb=tb):
                    def go():
                        ps = pp.tile([P, TB], f32, tag="mm512",
                                     name=f"yps_{co}_{tb}")
                        for ci in range(MCH):
                            nc.tensor.matmul(
                                ps,
                                lhsT=wot[:, ci * C + co * P:ci * C + (co + 1) * P],
                                rhs=ct[ci][:, tb * TB:(tb + 1) * TB],
                                start=(ci == 0), stop=(ci == MCH - 1))
                        ysb = yp.tile([P, TB], f32, tag="ysb", name=f"ysb_{co}_{tb}")
                        if copy_eng == "scalar":
                            nc.scalar.copy(ysb, ps)
                        else:
                            nc.vector.tensor_copy(ysb, ps)
                        nc.sync.dma_start(
                            out=yT[co * P:(co + 1) * P, tb * TB:(tb + 1) * TB],
                            in_=ysb)
                    return go
                return [out_co(co) for co in range(C // P)]

            pending = []
            st_rot = [0]

            def mk_norm(h, j, m, r0, ctx_ps):
                def norm():
                    s_sb = tmp.tile([1, TB], f32, tag="s", name=f"s_{h}_{j}")
                    nc.vector.tensor_copy(s_sb, ctx_ps[64:65, :])
                    r1 = tmp.tile([1, TB], f32, tag="r1", name=f"r1_{h}_{j}")
                    nc.vector.reciprocal_approx_fast(out=r1, in_=s_sb)
                    rb = tmp.tile([64, TB], f32, tag="rb", name=f"rb_{h}_{j}")
                    nc.gpsimd.partition_broadcast(rb, r1)
                    nc.vector.tensor_mul(
                        ct[m][r0:r0 + 64, j * TB:(j + 1) * TB], ctx_ps[0:64, :], rb)
                return norm

            pair_cnt = [0]

            def attend(j, ilq):
                nch = 4 * (j + 1)
                npair = nch // 2
                # hold back filler units to bridge the last head's
                # act->mask->ctx latency chain and the trailing norm flush
                # (no more attention matmuls remain to hide either)
                reserve = ilq[-3:]
                post = ilq[-6:-3]
                main = ilq[:max(0, len(ilq) - 6)]

                for h in range(HL):
                    last = h == HL - 1
                    m, r0 = h // 2, (h % 2) * 64
                    qs = qt[h][:, j * TB:(j + 1) * TB]
                    ctx_ps = cxp.tile([P, TB], f32, tag="ctx", name=f"cps_{h}_{j}")
                    inflight = []
                    nmm = [0]

                    def ctx_mms(pt_, pp0, ctx_ps=ctx_ps, h=h, nch=nch, j=j,
                                nmm=nmm):
                        for s in (0, 1):
                            c = 2 * pp0 + s
                            t_off = c - 4 * j
                            qoff = P * t_off if t_off > 0 else 0
                            nc.tensor.matmul(
                                ctx_ps[:, qoff:],
                                lhsT=v[c][:, h * P:(h + 1) * P],
                                rhs=pt_[:, s * TB + qoff:(s + 1) * TB],
                                start=(nmm[0] == 0), stop=(nmm[0] == nch - 1),
                                skip_group_check=True)
                            nmm[0] += 1

                    if h >= 1 and main:
                        main.pop(0)()
                    for pp_ in range(npair):
                        st = stt[st_rot[0] % 2]
                        st_rot[0] += 1
                        for s in (0, 1):
                            c = 2 * pp_ + s
                            t_off = c - 4 * j
                            qoff = P * t_off if t_off > 0 else 0
                            nc.tensor.matmul(
                                st[:, s * TB + qoff:(s + 1) * TB],
                                lhsT=kt[m][:, c * P:(c + 1) * P],
                                rhs=qs[:, qoff:],
                                start=True, stop=True, skip_group_check=True)
                        pt_ = ptp.tile([P, 2 * TB], bf16, tag="pt",
                                       name=f"pt_{h}_{j}_{pp_}")
                        # first valid column of the pair (s=0 chunk's trim)
                        t0_ = 2 * pp_ - 4 * j
                        a0 = P * t0_ if t0_ > 0 else 0
                        nc.scalar.activation(pt_[:, a0:], st[:, a0:], EXP,
                                             scale=SCALE)
                        for s in (0, 1):
                            c = 2 * pp_ + s
                            t_off = c - 4 * j
                            if t_off >= 0:
                                # boundary 128x128: keep q-k = f-p >= 0
                                blk = pt_[:, s * TB + P * t_off:
                                          s * TB + P * (t_off + 1)]
                                nc.gpsimd.affine_select(
                                    out=blk, in_=blk, compare_op=GE, fill=0.0,
                                    base=0, pattern=[[1, P]],
                                    channel_multiplier=-1)
                        if pp_ % 2 == 1 and pending:
                            pending.pop(0)()
                        inflight.append((pt_, pp_))
                        if len(inflight) > 2:
                            ctx_mms(*inflight.pop(0))
                        pair_cnt[0] += 1
                        if pair_cnt[0] % 6 == 2 and main:
                            main.pop(0)()
                    if last:
                        for g in main + reserve:
                            g()
                        main, reserve = [], []
                    dr = 0
                    while inflight:
                        if dr == 1 and not last and main:
                            main.pop(0)()
                        ctx_mms(*inflight.pop(0))
                        dr += 1
                    pending.append(mk_norm(h, j, m, r0, ctx_ps))
                for g in main + reserve:
                    g()
                if j == NTB - 1:
                    # only the final block must flush: out(3) consumes ct(3).
                    # earlier blocks' trailing norms drain inside the next
                    # attend (data deps are tracked; the next block's S
                    # matmuls don't depend on them).
                    while pending:
                        pending.pop(0)()
                for g in post:
                    g()

            for g in proj_groups(0):
                g()
            for tb in range(NTB):
                if tb < NTB - 1:
                    ilq = proj_groups(tb + 1)
                else:
                    ilq = out_groups(0) + out_groups(1) + out_groups(2)
                attend(tb, ilq)
            for g in out_groups(NTB - 1):
                g()

    nc.compile()
    return nc


def make_in_maps(x, Wq, Wk, Wv, Wo):
    import ml_dtypes
    bf = ml_dtypes.bfloat16

    x = np.asarray(x, np.float32)
    Wq, Wk, Wv, Wo = (np.asarray(w, np.float32) for w in (Wq, Wk, Wv, Wo))
    in_maps = []
    for core in range(NCORES):
        b, g = divmod(core, GROUPS)
        sl = slice(g * OC, (g + 1) * OC)
        xT = x[b].T                       # [C, T]
        xh = (xT.reshape(8, 128, 4, 512).transpose(1, 2, 0, 3)
              .reshape(128, 16384))
        wqT = Wq[sl, :].T                 # [C, OC]
        wqh = (wqT.reshape(8, 128, 4, 128).transpose(1, 2, 0, 3)
               .reshape(128, 4096))
        wkT = Wk[sl, :].T
        wkh = (wkT.reshape(8, 128, 4, 128).transpose(1, 2, 0, 3)
               .reshape(128, 4096))
        wvT = Wv[sl, :].T
        wvh = (wvT.reshape(8, 128, 512).transpose(1, 0, 2)
               .reshape(128, 4096))
        woT = Wo[:, sl].T                 # [OC, C]
        woh = (woT.reshape(4, 128, 1024).transpose(1, 0, 2)
               .reshape(128, 4096))
        in_maps.append({
            "xh": np.ascontiguousarray(xh).astype(bf),
            "wqh": np.ascontiguousarray(wqh).astype(bf),
            "wkh": np.ascontiguousarray(wkh).astype(bf),
            "wvh": np.ascontiguousarray(wvh).astype(bf),
            "woh": np.ascontiguousarray(woh).astype(bf),
        })
    return in_maps


def _run(inputs, trace=False):
    from concourse.bass_utils import run_bass_kernel_spmd

    nc = build_program()
    in_maps = make_in_maps(
        inputs["x"], inputs["Wq"], inputs["Wk"], inputs["Wv"], inputs["Wo"])
    res = run_bass_kernel_spmd(nc, in_maps, core_ids=list(range(NCORES)), trace=trace)
    y = np.zeros((B, T_FULL, C), np.float32)
    for core in range(NCORES):
        y[core // GROUPS] += res.results[core]["yT"].T
    return y, res


def kernel(**inputs):
    y, _ = _run(inputs)
    return y


# revision 28
# speedup vs baseline: 1.0156x; 1.0156x over previous
"""Causal multi-head attention (B=4, T=2048, C=1024, H=16, HD=64) on 8 TRN2
NeuronCores.

Sharding: 2D — batch (4) x head-group (2 groups of 8 heads). Each core handles
one batch's tokens for 8 heads:
    core = b * 2 + g
    OC = 512 local channels; host-side reduce over g of yT.T partials.

v2 layout/schedule (vs. v1 baseline at ~342us):
  - All on-chip operands bf16 (x, Wq/Wk/Wv/Wo, q/k/v, ctx). PSUM stays f32;
    y output stays f32. Halves input DMA (16->8 MB per core).
  - Host pre-tiles every input into the exact SBUF layout with 8 KB
    contiguous rows -> 8 consolidated input DMAs with full-size packets.
  - Causal trim: per query block j, diagonal key chunk c=4j+t only streams
    q in [128t, 512) through the S^T and ctx matmuls (saves ~20us of PE
    streams); masking is a single 128x128 affine_select per diagonal chunk
    (keep f - p >= 0) instead of full-tile masks.
  - Ascending chunk order so ctx PSUM accumulation starts with the full-width
    chunk 0 (partial-width chunks then accumulate into already-started cols).
  - Filler rebalance: scalar-engine exp (1114ns per chunk pair) outruns the
    PE's attention matmuls (864ns per pair), so attend(j) interleaves spare
    matmul groups: proj(j+1) for j=0..2 and ALL output projections out(0..2)
    during attend(3) (which has the largest exp deficit). Filler is paced one
    unit per ~3 pairs to spread it across the exp-limited stretch.
  - Q^T/K^T in [channel, token] bf16; V in [token, channel] bf16 with an
    all-ones column per head so the ctx matmul also produces the softmax
    denominator row; DVE reciprocal + gpsimd partition_broadcast + DVE mult
    for normalization (deferred via a pending queue).
"""

import numpy as np

B, T_FULL, C = 4, 2048, 1024
H, HD = 16, 64
GROUPS = 2
HL = H // GROUPS          # heads per core = 8
OC = HL * HD              # local channels = 512
P = 128                   # partitions
TB = 512                  # token block (moving dim)
SCALE = float(1.0 / np.sqrt(HD))
NCORES = 8


def build_program(T=T_FULL):
    from contextlib import ExitStack

    import concourse.bacc as bacc
    import concourse.mybir as mybir
    import concourse.tile as tile

    f32 = mybir.dt.float32
    bf16 = mybir.dt.bfloat16
    EXP = mybir.ActivationFunctionType.Exp
    GE = mybir.AluOpType.is_ge
    MULT = mybir.AluOpType.mult

    NTB = T // TB             # 512-token blocks = 4
    NKC = T // P              # 128-token key chunks = 16
    CCH = C // P              # 8 contraction chunks of C
    MCH = OC // P             # 4 output-channel chunks

    nc = bacc.Bacc("TRN2", target_bir_lowering=False, debug=False)
    # Host-pretiled inputs: row p holds the SBUF partition-p contents.
    xh = nc.dram_tensor("xh", [P, NTB * CCH * TB], bf16, kind="ExternalInput").ap()
    wqh = nc.dram_tensor("wqh", [P, MCH * CCH * P], bf16, kind="ExternalInput").ap()
    wkh = nc.dram_tensor("wkh", [P, MCH * CCH * P], bf16, kind="ExternalInput").ap()
    wvh = nc.dram_tensor("wvh", [P, CCH * OC], bf16, kind="ExternalInput").ap()
    woh = nc.dram_tensor("woh", [P, MCH * C], bf16, kind="ExternalInput").ap()
    yT = nc.dram_tensor("yT", [C, T], f32, kind="ExternalOutput").ap()

    with tile.TileContext(nc) as tc, ExitStack() as ctx:
        perm = ctx.enter_context(tc.tile_pool(name="perm", bufs=1))
        psperm = ctx.enter_context(tc.tile_pool(name="psperm", bufs=1, space="PSUM"))

        # persistent SBUF tiles
        xb = [perm.tile([P, CCH * TB], bf16, tag=f"xb{t}", name=f"xb{t}")
              for t in range(NTB)]
        wqt = perm.tile([P, MCH * CCH * P], bf16, tag="wqt", name="wqt")
        wkt = perm.tile([P, MCH * CCH * P], bf16, tag="wkt", name="wkt")
        wvt = perm.tile([P, CCH * OC], bf16, tag="wvt", name="wvt")
        wot = perm.tile([P, MCH * C], bf16, tag="wot", name="wot")
        qt = [perm.tile([P, T], bf16, tag=f"qt{h}", name=f"qt{h}") for h in range(HL)]
        kt = [perm.tile([P, T], bf16, tag=f"kt{m}", name=f"kt{m}") for m in range(MCH)]
        ct = [perm.tile([P, T], bf16, tag=f"ct{m}", name=f"ct{m}") for m in range(MCH)]
        # V padded to 128 cols per head: [V_h | 1 | 0...] so ctx lhsT is M=128
        v = [perm.tile([P, HL * P], bf16, tag=f"v{t}", name=f"v{t}")
             for t in range(NKC)]
        # S^T psum tiles, manually rotated (memset once so trimmed-pair
        # activations never read never-written PSUM)
        stt = [psperm.tile([P, 2 * TB], f32, tag=f"st{i}", name=f"st{i}")
               for i in range(2)]

        # ---- input DMAs, priority order (wq split per-m so the first
        # projection group is gated on only x0 + 256 KB of weights) ----
        MW = CCH * P
        HB = CCH * TB // 2
        nc.sync.dma_start(out=xb[0][:, 0:HB], in_=xh[:, 0:HB])
        nc.sync.dma_start(out=wqt[:, 0:MW], in_=wqh[:, 0:MW])
        nc.sync.dma_start(out=xb[0][:, HB:2 * HB], in_=xh[:, HB:2 * HB])
        for m in range(1, MCH):
            nc.sync.dma_start(out=wqt[:, m * MW:(m + 1) * MW],
                              in_=wqh[:, m * MW:(m + 1) * MW])
        nc.sync.dma_start(out=wkt, in_=wkh)
        nc.sync.dma_start(out=wvt, in_=wvh)
        for t in range(1, NTB):
            nc.sync.dma_start(out=xb[t], in_=xh[:, t * CCH * TB:(t + 1) * CCH * TB])
        nc.sync.dma_start(out=wot, in_=woh)

        # ---- one-time memsets, all on gpsimd (its first real work — the
        # first affine_select — comes well after these drain), ordered by
        # first use so nothing blocks the projection pipeline's DVE copies.
        ONE_BF16 = 0x3F80  # 1.0 in bf16 — bf16 memset via uint16 bitcast
        for st in stt:
            nc.vector.memset(st, 0.0)  # gpsimd memset cannot target PSUM

        def memset_v(vt):
            vv = vt.rearrange("p (h e) -> p h e", e=P)
            nc.gpsimd.memset(vv[:, :, 64:65].bitcast(mybir.dt.uint16), ONE_BF16)
            nc.gpsimd.memset(vv[:, :, 65:].bitcast(mybir.dt.uint16), 0)

        for vt in v[:4]:
            memset_v(vt)
        for h in range(HL):
            z0 = (1 - h % 2) * 64  # zero rows: the other head's half
            nc.gpsimd.memset(qt[h][z0:z0 + 64, :].bitcast(mybir.dt.uint16), 0)
        for vt in v[4:]:
            memset_v(vt)

        with (
            tc.tile_pool(name="ptpool", bufs=3) as ptp,
            tc.tile_pool(name="tmppool", bufs=2) as tmp,
            tc.tile_pool(name="ypool", bufs=4) as yp,
            tc.tile_pool(name="mmps", bufs=2, space="PSUM") as pp,
            tc.tile_pool(name="ctxps", bufs=2, space="PSUM") as cxp,
        ):
            def proj_groups(tb):
                groups = []

                def proj_qk(wt, isq, m, tb=tb, copy_eng=None):
                    def go():
                        ps = pp.tile([P, TB], f32, tag="mm512",
                                     name=f"ps_{tb}_{m}_{isq}")
                        for c in range(CCH):
                            nc.tensor.matmul(
                                ps,
                                lhsT=wt[:, m * (CCH * P) + c * P:
                                        m * (CCH * P) + (c + 1) * P],
                                rhs=xb[tb][:, c * TB:(c + 1) * TB],
                                start=(c == 0), stop=(c == CCH - 1))
                        cp = (nc.scalar.copy if copy_eng == "scalar"
                              else nc.vector.tensor_copy)
                        if isq:
                            for hh in (0, 1):
                                r0_ = hh * 64
                                cp(qt[2 * m + hh][r0_:r0_ + 64,
                                                  tb * TB:(tb + 1) * TB],
                                   ps[r0_:r0_ + 64, :])
                        else:
                            cp(kt[m][:, tb * TB:(tb + 1) * TB], ps)
                    return go

                def proj_v(ts_, tb=tb, copy_eng=None):
                    def go():
                        ps = pp.tile([P, OC], f32, tag="mm512",
                                     name=f"psv_{tb}_{ts_}")
                        for c in range(CCH):
                            nc.tensor.matmul(
                                ps,
                                lhsT=xb[tb][:, c * TB + ts_ * P:
                                            c * TB + (ts_ + 1) * P],
                                rhs=wvt[:, c * OC:(c + 1) * OC],
                                start=(c == 0), stop=(c == CCH - 1))
                        ti = tb * (TB // P) + ts_
                        cp = (nc.scalar.copy if copy_eng == "scalar"
                              else nc.vector.tensor_copy)
                        cp(v[ti].rearrange("p (h e) -> p h e", e=P)[:, :, 0:64],
                           ps.rearrange("p (h d) -> p h d", d=64))
                    return go

                # the last 6 units are consumed at the attend tail where the
                # DVE queue is congested with norms — put their psum->sbuf
                # copies on the scalar engine (drained of exps by then)
                for m in range(MCH):
                    groups.append(proj_qk(wqt, True, m))
                for m in range(MCH):
                    groups.append(proj_qk(wkt, False, m,
                                          copy_eng="scalar" if m >= 2 else None))
                for ts_ in range(TB // P):
                    groups.append(proj_v(ts_, copy_eng="scalar"))
                return groups

            def out_groups(tb, copy_eng=None):
                def out_co(co, tb=tb):
                    def go():
                        ps = pp.tile([P, TB], f32, tag="mm512",
                                     name=f"yps_{co}_{tb}")
                        for ci in range(MCH):
                            nc.tensor.matmul(
                                ps,
                                lhsT=wot[:, ci * C + co * P:ci * C + (co + 1) * P],
                                rhs=ct[ci][:, tb * TB:(tb + 1) * TB],
                                start=(ci == 0), stop=(ci == MCH - 1))
                        ysb = yp.tile([P, TB], f32, tag="ysb", name=f"ysb_{co}_{tb}")
                        if copy_eng == "scalar":
                            nc.scalar.copy(ysb, ps)
                        else:
                            nc.vector.tensor_copy(ysb, ps)
                        nc.sync.dma_start(
                            out=yT[co * P:(co + 1) * P, tb * TB:(tb + 1) * TB],
                            in_=ysb)
                    return go
                return [out_co(co) for co in range(C // P)]

            pending = []
            st_rot = [0]

            def mk_norm(h, j, m, r0, ctx_ps):
                def norm():
                    s_sb = tmp.tile([1, TB], f32, tag="s", name=f"s_{h}_{j}")
                    nc.vector.tensor_copy(s_sb, ctx_ps[64:65, :])
                    r1 = tmp.tile([1, TB], f32, tag="r1", name=f"r1_{h}_{j}")
                    nc.vector.reciprocal_approx_fast(out=r1, in_=s_sb)
                    rb = tmp.tile([64, TB], f32, tag="rb", name=f"rb_{h}_{j}")
                    nc.gpsimd.partition_broadcast(rb, r1)
                    nc.vector.tensor_mul(
                        ct[m][r0:r0 + 64, j * TB:(j + 1) * TB], ctx_ps[0:64, :], rb)
                return norm

            pair_cnt = [0]

            def attend(j, ilq):
                nch = 4 * (j + 1)
                npair = nch // 2
                # hold back filler units to bridge the last head's
                # act->mask->ctx latency chain and the trailing norm flush
                # (no more attention matmuls remain to hide either)
                reserve = ilq[-3:]
                post = ilq[-6:-3]
                main = ilq[:max(0, len(ilq) - 6)]

                for h in range(HL):
                    last = h == HL - 1
                    m, r0 = h // 2, (h % 2) * 64
                    qs = qt[h][:, j * TB:(j + 1) * TB]
                    ctx_ps = cxp.tile([P, TB], f32, tag="ctx", name=f"cps_{h}_{j}")
                    inflight = []
                    nmm = [0]

                    def ctx_mms(pt_, pp0, ctx_ps=ctx_ps, h=h, nch=nch, j=j,
                                nmm=nmm):
                        for s in (0, 1):
                            c = 2 * pp0 + s
                            t_off = c - 4 * j
                            qoff = P * t_off if t_off > 0 else 0
                            nc.tensor.matmul(
                                ctx_ps[:, qoff:],
                                lhsT=v[c][:, h * P:(h + 1) * P],
                                rhs=pt_[:, s * TB + qoff:(s + 1) * TB],
                                start=(nmm[0] == 0), stop=(nmm[0] == nch - 1),
                                skip_group_check=True)
                            nmm[0] += 1

                    if h >= 1 and main:
                        main.pop(0)()
                    for pp_ in range(npair):
                        st = stt[st_rot[0] % 2]
                        st_rot[0] += 1
                        for s in (0, 1):
                            c = 2 * pp_ + s
                            t_off = c - 4 * j
                            qoff = P * t_off if t_off > 0 else 0
                            nc.tensor.matmul(
                                st[:, s * TB + qoff:(s + 1) * TB],
                                lhsT=kt[m][:, c * P:(c + 1) * P],
                                rhs=qs[:, qoff:],
                                start=True, stop=True, skip_group_check=True)
                        pt_ = ptp.tile([P, 2 * TB], bf16, tag="pt",
                                       name=f"pt_{h}_{j}_{pp_}")
                        # first valid column of the pair (s=0 chunk's trim)
                        t0_ = 2 * pp_ - 4 * j
                        a0 = P * t0_ if t0_ > 0 else 0
                        nc.scalar.activation(pt_[:, a0:], st[:, a0:], EXP,
                                             scale=SCALE)
                        for s in (0, 1):
                            c = 2 * pp_ + s
                            t_off = c - 4 * j
                            if t_off >= 0:
                                # boundary 128x128: keep q-k = f-p >= 0
                                blk = pt_[:, s * TB + P * t_off:
                                          s * TB + P * (t_off + 1)]
                                nc.gpsimd.affine_select(
                                    out=blk, in_=blk, compare_op=GE, fill=0.0,
                                    base=0, pattern=[[1, P]],
                                    channel_multiplier=-1)
                        if pp_ % 2 == 1 and pending:
                            pending.pop(0)()
                        inflight.append((pt_, pp_))
                        if len(inflight) > 2:
                            ctx_mms(*inflight.pop(0))
                        pair_cnt[0] += 1
                        if pair_cnt[0] % 6 == 2 and main:
                            main.pop(0)()
                    if last:
                        for g in main + reserve:
                            g()
                        main, reserve = [], []
                    dr = 0
                    while inflight:
                        if dr == 1 and not last and main:
                            main.pop(0)()
                        ctx_mms(*inflight.pop(0))
                        dr += 1
                    pending.append(mk_norm(h, j, m, r0, ctx_ps))
                for g in main + reserve:
                    g()
                if j == NTB - 1:
                    # only the final block must flush: out(3) consumes ct(3).
                    # earlier blocks' trailing norms drain inside the next
                    # attend (data deps are tracked; the next block's S
                    # matmuls don't depend on them).
                    while pending:
                        pending.pop(0)()
                for g in post:
                    g()

            for g in proj_groups(0):
                g()
            for tb in range(NTB):
                if tb < NTB - 1:
                    ilq = proj_groups(tb + 1)
                else:
                    # out(2) units are consumed last, when the scalar engine
                    # has drained its exps — route their PSUM->SBUF copies
                    # there to keep the congested DVE queue out of the
                    # pp-pool rotation's critical path.
                    ilq = (out_groups(0) + out_groups(1)
                           + out_groups(2, copy_eng="scalar"))
                attend(tb, ilq)
            for g in out_groups(NTB - 1):
                g()

    nc.compile()
    return nc


def make_in_maps(x, Wq, Wk, Wv, Wo):
    import ml_dtypes
    bf = ml_dtypes.bfloat16

    x = np.asarray(x, np.float32)
    Wq, Wk, Wv, Wo = (np.asarray(w, np.float32) for w in (Wq, Wk, Wv, Wo))
    in_maps = []
    for core in range(NCORES):
        b, g = divmod(core, GROUPS)
        sl = slice(g * OC, (g + 1) * OC)
        xT = x[b].T                       # [C, T]
        xh = (xT.reshape(8, 128, 4, 512).transpose(1, 2, 0, 3)
              .reshape(128, 16384))
        wqT = Wq[sl, :].T                 # [C, OC]
        wqh = (wqT.reshape(8, 128, 4, 128).transpose(1, 2, 0, 3)
               .reshape(128, 4096))
        wkT = Wk[sl, :].T
        wkh = (wkT.reshape(8, 128, 4, 128).transpose(1, 2, 0, 3)
               .reshape(128, 4096))
        wvT = Wv[sl, :].T
        wvh = (wvT.reshape(8, 128, 512).transpose(1, 0, 2)
               .reshape(128, 4096))
        woT = Wo[:, sl].T                 # [OC, C]
        woh = (woT.reshape(4, 128, 1024).transpose(1, 0, 2)
               .reshape(128, 4096))
        in_maps.append({
            "xh": np.ascontiguousarray(xh).astype(bf),
            "wqh": np.ascontiguousarray(wqh).astype(bf),
            "wkh": np.ascontiguousarray(wkh).astype(bf),
            "wvh": np.ascontiguousarray(wvh).astype(bf),
            "woh": np.ascontiguousarray(woh).astype(bf),
        })
    return in_maps


def _run(inputs, trace=False):
    from concourse.bass_utils import run_bass_kernel_spmd

    nc = build_program()
    in_maps = make_in_maps(
        inputs["x"], inputs["Wq"], inputs["Wk"], inputs["Wv"], inputs["Wo"])
    res = run_bass_kernel_spmd(nc, in_maps, core_ids=list(range(NCORES)), trace=trace)
    y = np.zeros((B, T_FULL, C), np.float32)
    for core in range(NCORES):
        y[core // GROUPS] += res.results[core]["yT"].T
    return y, res


def kernel(**inputs):
    y, _ = _run(inputs)
    return y
